# revision 4
# baseline (speedup 1.0000x reference)
"""Trainium2 Bass kernel v3 for nn_NeSyBase_36575941493336 (moe_routing).

Changes vs v2 baseline (2205us claimed / ~2360us measured):

D1 (projections): ldweights amortized (k-loop outer, 4 psum banks inner) and
    output DMAs restructured so every HBM segment is a contiguous 16KB block
    (was 128-byte bursts through a 't g s -> g t s' rearrange).

D2 (recurrence): Wh in fp8e4 scaled by 16 (FWL makes ldweights ~4x faster;
    measured pure-mm step 5.8us -> 2.7us), descale folded into the activation
    scale. All four gate-group psums are preloaded with xg via a single
    identity matmul each (removes all DVE adds from the critical path; the
    preloads depend only on the xg DMA so they overlap the previous step's
    elementwise tail). Gates + c-chain in fp16 (DVE 2x mode).

Host MLP tail unchanged (0.06% of FLOPs).
"""

import numpy as np
import ml_dtypes

import concourse.bacc as bacc
import concourse.mybir as mybir
import concourse.tile as tile
from concourse.bass_utils import run_bass_kernel_spmd

FP16 = mybir.dt.float16
FP8 = mybir.dt.float8e4
FP32 = mybir.dt.float32

SEQ = 128          # B*S sequences
TV = 256           # video timesteps
TT = 64            # text timesteps
DV = 1024          # video input dim (2E)
DT = 512           # text input dim (E)
H = 512            # hidden
G = 2048           # 4H gates
SH = 64            # seqs per half
SC = 16.0          # fp8 weight scale (exact power of two)
ACT = mybir.ActivationFunctionType


def _new_nc():
    return bacc.Bacc("TRN2", target_bir_lowering=False)


# --------------------------------------------------------------------------
# D1: input projections (ld-amortized, contiguous output DMA)
# --------------------------------------------------------------------------

def build_d1(tqv=TV // 4, tqt=TT // 4):
    """fp8e4 x and Wx with DoubleRow matmuls (K=256 per instruction)."""
    nc = _new_nc()
    DR = mybir.MatmulPerfMode.DoubleRow
    xtv = nc.declare_dram_parameter("xtv", [DV, tqv * SH], FP8, isOutput=False)
    xtt = nc.declare_dram_parameter("xtt", [DT, tqt * SH], FP8, isOutput=False)
    wvf = nc.declare_dram_parameter("wvf", [DV, G], FP8, isOutput=False)
    wvb = nc.declare_dram_parameter("wvb", [DV, G], FP8, isOutput=False)
    wtf = nc.declare_dram_parameter("wtf", [DT, G], FP8, isOutput=False)
    wtb = nc.declare_dram_parameter("wtb", [DT, G], FP8, isOutput=False)
    # Output layout [m, n, p, t*s]: each (m, n) DMA is one contiguous 128KB
    # write (the host repacks to the [t, p, m, s] layout D2 streams from).
    nv, nt = tqv * SH // 512, tqt * SH // 512
    xgvf = nc.declare_dram_parameter("xgvf", [16, nv, 128, 512], FP16, isOutput=True)
    xgvb = nc.declare_dram_parameter("xgvb", [16, nv, 128, 512], FP16, isOutput=True)
    xgtf = nc.declare_dram_parameter("xgtf", [16, nt, 128, 512], FP16, isOutput=True)
    xgtb = nc.declare_dram_parameter("xgtb", [16, nt, 128, 512], FP16, isOutput=True)

    with tile.TileContext(nc) as tc:
        with (
            tc.tile_pool(name="xin", bufs=1) as xin,
            tc.tile_pool(name="win", bufs=2) as win,
            tc.tile_pool(name="ps", bufs=2, space="PSUM") as psp,
            tc.tile_pool(name="out", bufs=8) as outp,
        ):
            # [p, kpair, 2, cols]: dim '2' selects the two K=128 halves of a
            # 256-row DoubleRow block.
            xv_sb = xin.tile([128, DV // 256, 2, tqv * SH], FP8, tag="xv")
            nc.sync.dma_start(
                out=xv_sb, in_=xtv.rearrange("(kt two p) n -> p kt two n", p=128, two=2)
            )
            xt_sb = xin.tile([128, DT // 256, 2, tqt * SH], FP8, tag="xt")
            nc.sync.dma_start(
                out=xt_sb, in_=xtt.rearrange("(kt two p) n -> p kt two n", p=128, two=2)
            )

            def project(w_dram, x_sb, xg_dram, kt2, ncols, wtag):
                w_sb = win.tile([128, kt2, 2, G], FP8, tag=wtag, name=f"w_{wtag}")
                nc.sync.dma_start(
                    out=w_sb,
                    in_=w_dram.rearrange("(kt two p) g -> p kt two g", p=128, two=2),
                )
                nchunk = ncols // 512
                ngrp = 4 if nchunk >= 4 else nchunk
                for m in range(16):
                    for n0 in range(0, nchunk, ngrp):
                        pss = [
                            psp.tile([128, 512], FP32, tag=f"ps{j}", name=f"ps{j}")
                            for j in range(ngrp)
                        ]
                        for k in range(kt2):
                            for j in range(ngrp):
                                n = n0 + j
                                nc.tensor.matmul(
                                    pss[j],
                                    lhsT=w_sb[:, k, :, 128 * m : 128 * (m + 1)],
                                    rhs=x_sb[:, k, :, 512 * n : 512 * (n + 1)],
                                    start=(k == 0),
                                    stop=(k == kt2 - 1),
                                    perf_mode=DR,
                                )
                        for j in range(ngrp):
                            n = n0 + j
                            ot = outp.tile([128, 512], FP16, tag="ot", name="ot")
                            if j % 2 == 0:
                                nc.vector.tensor_copy(ot, pss[j])
                            else:
                                nc.scalar.copy(out=ot, in_=pss[j])
                            nc.sync.dma_start(out=xg_dram[m, n], in_=ot)

            project(wvf, xv_sb, xgvf, DV // 256, tqv * SH, "wv")
            project(wvb, xv_sb, xgvb, DV // 256, tqv * SH, "wv")
            project(wtf, xt_sb, xgtf, DT // 256, tqt * SH, "wt")
            project(wtb, xt_sb, xgtb, DT // 256, tqt * SH, "wt")

    nc.compile()
    return nc


# --------------------------------------------------------------------------
# D2: LSTM recurrence, fp8 Wh + xg psum-preload via identity matmuls
# --------------------------------------------------------------------------

def build_d2(T=TV, B=SH):
    nc = _new_nc()
    wh = nc.declare_dram_parameter("wh", [H, G], FP8, isOutput=False)
    # xg layout [t, p, m, s]: per-step read is 2KB contiguous per partition.
    xg = nc.declare_dram_parameter("xg", [T, 128, 16, B], FP16, isOutput=False)
    ident = nc.declare_dram_parameter("ident", [128, 128], FP16, isOutput=False)
    hout = nc.declare_dram_parameter("hout", [128, (H // 128) * B], FP32, isOutput=True)

    KT = H // 128
    BL = (H // 128) * B
    GB = 4 * B
    dsc = 1.0 / SC

    with tile.TileContext(nc) as tc:
        with (
            tc.tile_pool(name="w", bufs=1) as wp,
            tc.tile_pool(name="xg", bufs=6) as xgp,
            tc.tile_pool(name="ps", bufs=2, space="PSUM") as psp,
            tc.tile_pool(name="ew", bufs=3) as ewp,
            tc.tile_pool(name="st", bufs=3) as stp,
        ):
            wh_sb = wp.tile([128, KT, G], FP8, tag="wh")
            nc.sync.dma_start(out=wh_sb, in_=wh.rearrange("(kt p) g -> p kt g", p=128))
            id_sb = wp.tile([128, 128], FP16, tag="id")
            nc.sync.dma_start(out=id_sb, in_=ident[:, :])

            hT = stp.tile([128, BL], FP16, tag="h")
            nc.vector.memset(hT, 0.0)
            cT = stp.tile([128, BL], FP16, tag="c")
            nc.vector.memset(cT, 0.0)

            def mm_group(ps, mlo, mhi, h_rhs):
                for j in range(mhi - mlo):
                    m = mlo + j
                    for k in range(KT):
                        nc.tensor.matmul(
                            ps[:, B * j : B * (j + 1)],
                            lhsT=wh_sb[:, k, 128 * m : 128 * (m + 1)],
                            rhs=h_rhs[:, B * k : B * (k + 1)],
                            start=False,
                            stop=(k == KT - 1),
                        )

            for t in range(T):
                xg_t = xgp.tile([128, 16, B], FP16, tag="xg")
                nc.sync.dma_start(out=xg_t, in_=xg[t])

                def xg_sl(mlo, mhi):
                    return xg_t[:, mlo:mhi, :].rearrange("p m s -> p (m s)")

                # psG/psO padded to a full 512-f32 bank: has_written clearing
                # on start=True is bank-wide, so each bank gets exactly ONE
                # start=True (its xg preload) and nothing else may share it.
                psG = psp.tile([128, 2 * GB], FP32, tag="psG", name="psG")[:, :GB]
                psIF = psp.tile([128, 2 * GB], FP32, tag="psIF", name="psIF")
                psO = psp.tile([128, 2 * GB], FP32, tag="psO", name="psO")[:, :GB]

                # xg preloads: depend only on the xg DMA, so they overlap the
                # previous step's elementwise tail on the PE.
                nc.tensor.matmul(psG, lhsT=id_sb, rhs=xg_sl(0, 4),
                                 start=True, stop=False)
                nc.tensor.matmul(psIF, lhsT=id_sb, rhs=xg_sl(4, 12),
                                 start=True, stop=False)
                nc.tensor.matmul(psO, lhsT=id_sb, rhs=xg_sl(12, 16),
                                 start=True, stop=False)

                h_prev = hT
                mm_group(psG, 0, 4, h_prev)
                tanh_g = ewp.tile([128, GB], FP16, tag="tanh_g")
                nc.scalar.activation(tanh_g, psG, ACT.Tanh, scale=dsc)

                mm_group(psIF[:, :GB], 4, 8, h_prev)
                mm_group(psIF[:, GB:], 8, 12, h_prev)
                sig_if = ewp.tile([128, 2 * GB], FP16, tag="sig_if")
                nc.scalar.activation(sig_if, psIF, ACT.Sigmoid, scale=dsc)

                mm_group(psO, 12, 16, h_prev)

                ig = ewp.tile([128, BL], FP16, tag="ig")
                nc.vector.tensor_mul(ig, sig_if[:, :GB], tanh_g)
                fc = ewp.tile([128, BL], FP16, tag="fc")
                nc.vector.tensor_mul(fc, sig_if[:, GB:], cT)
                cT = stp.tile([128, BL], FP16, tag="c")
                nc.vector.tensor_add(cT, fc, ig)
                tanh_c = ewp.tile([128, BL], FP16, tag="tanh_c")
                nc.scalar.activation(tanh_c, cT, ACT.Tanh)

                sig_o = ewp.tile([128, GB], FP16, tag="sig_o")
                nc.scalar.activation(sig_o, psO, ACT.Sigmoid, scale=dsc)

                hT = stp.tile([128, BL], FP16, tag="h")
                nc.vector.tensor_mul(hT, sig_o, tanh_c)

                if t == T - 1:
                    hF = stp.tile([128, BL], FP32, tag="hf")
                    nc.vector.tensor_mul(hF, sig_o, tanh_c)
                    nc.sync.dma_start(out=hout[:, :], in_=hF)

    nc.compile()
    return nc


# --------------------------------------------------------------------------
# Host orchestration
# --------------------------------------------------------------------------

def permute_gates(w):
    """[.., 4H] in torch order [i|f|g|o] -> kernel order [g|i|f|o]."""
    i, f, g, o = np.split(np.asarray(w), 4, axis=-1)
    return np.concatenate([g, i, f, o], axis=-1)


def _prep_d1_inputs(vid, txt, wxvf, wxvb, wxtf, wxtb):
    tqv, tqt = TV // 4, TT // 4
    f8 = ml_dtypes.float8_e4m3
    # Wx scaled by SC so xg comes out of D1 already scaled for the fp8 path.
    w8 = {
        "wvf": (permute_gates(wxvf) * SC).astype(f8),
        "wvb": (permute_gates(wxvb) * SC).astype(f8),
        "wtf": (permute_gates(wxtf) * SC).astype(f8),
        "wtb": (permute_gates(wxtb) * SC).astype(f8),
    }
    in_maps = []
    for c in range(8):
        q, h = c % 4, c // 4
        cv = vid[SH * h : SH * (h + 1), tqv * q : tqv * (q + 1), :]
        ct = txt[SH * h : SH * (h + 1), tqt * q : tqt * (q + 1), :]
        xtv = np.ascontiguousarray(cv.transpose(2, 1, 0)).reshape(DV, -1)
        xtt = np.ascontiguousarray(ct.transpose(2, 1, 0)).reshape(DT, -1)
        in_maps.append({"xtv": xtv.astype(f8), "xtt": xtt.astype(f8), **w8})
    return in_maps


_IDENT = np.eye(128, dtype=np.float16)


def _quarter_to_tpms(arr):
    """[16m, n, 128p, 512(t s)] -> [n*8 t, 128p, 16m, 64s]."""
    m, n = arr.shape[0], arr.shape[1]
    a = arr.reshape(m, n, 128, 512 // SH, SH)
    a = a.transpose(1, 3, 2, 0, 4)
    return np.ascontiguousarray(a).reshape(n * (512 // SH), 128, m, SH)


def _assemble_d2_inputs(d1_results, whvf, whvb, whtf, whtb):
    def cat(key, h):
        return np.concatenate(
            [_quarter_to_tpms(d1_results[h * 4 + q][key]) for q in range(4)], axis=0
        )

    pad = np.zeros((TV - TT, 128, 16, SH), np.float16)
    whvf, whvb = permute_gates(whvf), permute_gates(whvb)
    whtf, whtb = permute_gates(whtf), permute_gates(whtb)
    whs = {0: whvf, 1: whvf, 2: whvb, 3: whvb, 4: whtf, 5: whtf, 6: whtb, 7: whtb}
    in_maps = []
    for c in range(8):
        h = c % 2
        if c < 2:
            xg_full = cat("xgvf", h)
        elif c < 4:
            xg_full = cat("xgvb", h)[::-1]
        elif c < 6:
            xg_full = np.concatenate([pad, cat("xgtf", h)], axis=0)
        else:
            xg_full = np.concatenate([pad, cat("xgtb", h)[::-1]], axis=0)
        in_maps.append({
            "wh": (whs[c] * SC).astype(ml_dtypes.float8_e4m3),
            "xg": np.ascontiguousarray(xg_full),
            "ident": _IDENT,
        })
    return in_maps


def _assemble_feats(d2_results):
    feats = np.zeros((SEQ, 4 * H), np.float32)
    for c in range(8):
        h = c % 2
        d = c // 2
        hT = d2_results[c]["hout"]
        for k in range(H // 128):
            blk = hT[:, SH * k : SH * (k + 1)]
            feats[
                SH * h : SH * (h + 1), d * H + 128 * k : d * H + 128 * (k + 1)
            ] = blk.T
    return feats


_CACHE = {}
LAST_PHASE_TIMES = {}
LAST_IN_MAPS = {}


def kernel(**inputs) -> np.ndarray:
    import time

    if "d1" not in _CACHE:
        _CACHE["d1"] = build_d1()
        _CACHE["d2"] = build_d2()
    d1_nc, d2_nc = _CACHE["d1"], _CACHE["d2"]

    vid = np.asarray(inputs["vid_feats"], np.float32).reshape(SEQ, TV, DV)
    txt = np.asarray(inputs["text_feats"], np.float32).reshape(SEQ, TT, DT)

    for bname in ("vid_b_f", "vid_b_b", "txt_b_f", "txt_b_b"):
        assert not np.any(np.asarray(inputs[bname])), (
            f"nonzero LSTM bias {bname} not supported"
        )

    t0 = time.time()
    d1_in = _prep_d1_inputs(
        vid, txt,
        np.asarray(inputs["vid_Wx_f"]), np.asarray(inputs["vid_Wx_b"]),
        np.asarray(inputs["txt_Wx_f"]), np.asarray(inputs["txt_Wx_b"]),
    )
    LAST_PHASE_TIMES["prep_d1"] = time.time() - t0
    LAST_IN_MAPS["d1"] = d1_in

    t0 = time.time()
    r1 = run_bass_kernel_spmd(d1_nc, d1_in, list(range(8)))
    LAST_PHASE_TIMES["d1"] = time.time() - t0

    t0 = time.time()
    d2_in = _assemble_d2_inputs(
        r1.results,
        np.asarray(inputs["vid_Wh_f"]), np.asarray(inputs["vid_Wh_b"]),
        np.asarray(inputs["txt_Wh_f"]), np.asarray(inputs["txt_Wh_b"]),
    )
    LAST_PHASE_TIMES["prep_d2"] = time.time() - t0
    LAST_IN_MAPS["d2"] = d2_in

    t0 = time.time()
    r2 = run_bass_kernel_spmd(d2_nc, d2_in, list(range(8)))
    LAST_PHASE_TIMES["d2"] = time.time() - t0

    t0 = time.time()
    feats = _assemble_feats(r2.results)

    def mlp(W1, b1, W2, b2):
        h1 = np.maximum(
            feats @ np.asarray(W1, np.float32) + np.asarray(b1, np.float32), 0.0
        )
        return (h1 @ np.asarray(W2, np.float32) + np.asarray(b2, np.float32))[:, 0]

    state = mlp(inputs["sq_W1"], inputs["sq_b1"], inputs["sq_W2"], inputs["sq_b2"])
    rel = mlp(inputs["rq_W1"], inputs["rq_b1"], inputs["rq_W2"], inputs["rq_b2"])
    labels = np.asarray(inputs["segment_labels"]).reshape(SEQ)
    sel = np.where(labels <= 3, state, rel).reshape(16, 8)
    out = (1.0 / (1.0 + np.exp(-sel.mean(axis=1)))).astype(np.float32)
    LAST_PHASE_TIMES["tail"] = time.time() - t0
    return out
